# revision 39
# baseline (speedup 1.0000x reference)
"""Trainium2 Bass kernel for nn_DiseaseHead.

Computation (per the reference):
    w_rs = weights.sum(-1)                       # [P, C]
    pw   = features * w_rs + bias                # [B, P, C]
    pwn  = LayerNorm_c(pw) * gamma + beta
    h    = relu(features + pwn)
    out  = einsum("bpc,dc->bpd", h, lin_w) + lin_b

Sharding: data-parallel over batch B across 8 NeuronCores; all small
parameters replicated.  Device layout puts batch rows on SBUF partitions
(128 at a time) and groups PG=5 points per vector op so the free dim is
PG*C=1280.  LN stats via bn_stats/bn_aggr; normalize+residual+relu fused
into tensor_scalar + tensor_tensor + ACT relu; the D=5 projection runs on
the tensor engine after a PE transpose of h.
"""

import sys

if "/opt/trn_rl_repo" not in sys.path:
    sys.path.insert(0, "/opt/trn_rl_repo")

import numpy as np

B, P, C, D = 8192, 25, 256, 5
NCORES = 8
BS = B // NCORES          # 1024 batch rows per core
PG = 5                    # points per group
NPG = P // PG             # 5 groups
NBT = BS // 128           # 8 batch tiles of 128 rows
LN_EPS = 1e-5

_cache = {}


def _build(has_gamma: bool, has_beta: bool, has_linb: bool):
    import concourse.bass as bass
    import concourse.tile as tile
    from concourse import bacc, mybir
    from concourse.masks import make_identity

    f32 = mybir.dt.float32
    Alu = mybir.AluOpType
    Act = mybir.ActivationFunctionType

    nc = bacc.Bacc("TRN2", target_bir_lowering=False, debug=False,
                   num_devices=NCORES)

    x = nc.dram_tensor("x", [P, BS, C], f32, kind="ExternalInput")
    w = nc.dram_tensor("w", [P * C, C], f32, kind="ExternalInput")
    bi = nc.dram_tensor("bi", [P, C], f32, kind="ExternalInput")
    g = nc.dram_tensor("g", [C], f32, kind="ExternalInput")
    bt_ = nc.dram_tensor("bt", [C], f32, kind="ExternalInput")
    lw = nc.dram_tensor("lw", [D, C], f32, kind="ExternalInput")
    lb = nc.dram_tensor("lb", [D], f32, kind="ExternalInput")
    y = nc.dram_tensor("y", [D, P, BS], f32, kind="ExternalOutput")

    with tile.TileContext(nc) as tc:
        with (
            tc.tile_pool(name="consts", bufs=1) as consts,
            tc.tile_pool(name="wred", bufs=3) as wred,
            tc.tile_pool(name="bcast", bufs=2) as bcast,
            tc.tile_pool(name="feat", bufs=5) as feat,
            tc.tile_pool(name="work", bufs=5) as work,
            tc.tile_pool(name="small", bufs=10) as small,
            tc.tile_pool(name="hT", bufs=8) as hTp,
            tc.tile_pool(name="stage", bufs=6) as stagep,
            tc.tile_pool(name="ps", bufs=5, space="PSUM") as psp,
            tc.tile_pool(name="pso", bufs=3, space="PSUM") as psop,
            tc.tile_pool(name="dram", bufs=1, space="DRAM") as dramp,
        ):
            ident = consts.tile([128, 128], f32)
            make_identity(nc, ident[:])

            eps_t = consts.tile([128, 1], f32)
            nc.vector.memset(eps_t[:], LN_EPS)

            # lin_w^T halves: [K=c(128), M=d] stationaries
            lwT0 = consts.tile([128, D], f32)
            lwT1 = consts.tile([128, D], f32)
            nc.gpsimd.dma_start(
                out=lwT0[:], in_=bass.AP(lw, 0, [[1, 128], [C, D]]))
            nc.gpsimd.dma_start(
                out=lwT1[:], in_=bass.AP(lw, 128, [[1, 128], [C, D]]))

            if has_linb:
                lbt = consts.tile([D, 1], f32)
                nc.gpsimd.dma_start(out=lbt[:], in_=bass.AP(lb, 0, [[1, D], [1, 1]]))

            if has_gamma:
                gB = consts.tile([128, C], f32)
                nc.gpsimd.dma_start(out=gB[:], in_=bass.AP(g, 0, [[0, 128], [1, C]]))
            if has_beta:
                bB2 = consts.tile([128, C], f32)
                nc.gpsimd.dma_start(out=bB2[:], in_=bass.AP(bt_, 0, [[0, 128], [1, C]]))

            # ---- weights row-sum on device: w_rs[p, c] = sum_k w[p, c, k] ----
            NW = (P * C) // 128  # 50 row-tiles, loaded 10 at a time
            wcols = consts.tile([128, NW], f32)
            for j in range(0, NW, 10):
                wt = wred.tile([128, 10, C], f32)
                nc.default_dma_engine.dma_start(
                    out=wt[:],
                    in_=bass.AP(w, j * 128 * C,
                                [[C, 128], [128 * C, 10], [1, C]]))
                nc.vector.tensor_reduce(
                    out=wcols[:, j:j + 10], in_=wt[:],
                    axis=mybir.AxisListType.X, op=Alu.add)
            # transpose [128, NW] -> [NW, 128] and park in DRAM so stride-0
            # broadcast DMAs can replicate rows across partitions
            wrs_ps = psp.tile([NW, 128], f32, tag="tb")
            nc.tensor.transpose(wrs_ps[:], wcols[:], ident[:])
            wrs_sb = consts.tile([NW, 128], f32)
            nc.vector.tensor_copy(wrs_sb[:], wrs_ps[:])
            wrs_d = dramp.tile([NW * 128], f32)   # flat == [P, C] row-major
            nc.gpsimd.dma_start(out=wrs_d[:], in_=wrs_sb[:])
            wrs_ap = wrs_d[:]

            # ---- main loop: point-groups outer, batch-tiles inner ----
            for pg in range(NPG):
                p0 = pg * PG
                # broadcast tiles for this point group: [128, PG, C]
                wB = bcast.tile([128, PG, C], f32, tag="wB")
                nc.gpsimd.dma_start(
                    out=wB[:],
                    in_=bass.AP(wrs_ap.tensor, wrs_ap.offset + p0 * C,
                                [[0, 128], [C, PG], [1, C]]))
                bB = bcast.tile([128, PG, C], f32, tag="bB")
                nc.gpsimd.dma_start(
                    out=bB[:],
                    in_=bass.AP(bi, p0 * C, [[0, 128], [C, PG], [1, C]]))
                # per-point bias sums for the mean correction: sum_c bias / C
                sumbC = bcast.tile([128, PG], f32, tag="sumbC")
                nc.vector.tensor_reduce(
                    out=sumbC[:], in_=bB[:], axis=mybir.AxisListType.X,
                    op=Alu.add)
                nc.vector.tensor_scalar_mul(sumbC[:], sumbC[:], 1.0 / C)

                for ibt in range(NBT):
                    b0 = ibt * 128
                    f = feat.tile([128, PG, C], f32, tag="f")
                    nc.default_dma_engine.dma_start(
                        out=f[:],
                        in_=bass.AP(x, (p0 * BS + b0) * C,
                                    [[C, 128], [BS * C, PG], [1, C]]))

                    # pw1 = f * w_rs with s1' = sum_c(f*w_rs) accumulated on
                    # the multiply (DVE); bias-add on GPSIMD; s2 = sum_c pw^2
                    # via square passes split across ACT and DVE.
                    pw1 = work.tile([128, PG, C], f32, tag="pw1")
                    s1 = small.tile([128, PG], f32, tag="s1")
                    s2 = small.tile([128, PG], f32, tag="s2")
                    for ip in range(PG):
                        nc.vector.scalar_tensor_tensor(
                            out=pw1[:, ip, :], in0=f[:, ip, :], scalar=1.0,
                            in1=wB[:, ip, :], op0=Alu.bypass, op1=Alu.mult,
                            accum_out=s1[:, ip:ip + 1])
                    pw = work.tile([128, PG, C], f32, tag="pw")
                    nc.gpsimd.tensor_add(pw[:], pw1[:], bB[:])
                    # squares (for the second moment) overwrite dead pw1
                    for ip in range(PG):
                        if ip < 2:
                            nc.scalar.activation(
                                pw1[:, ip, :], pw[:, ip, :], Act.Square,
                                accum_out=s2[:, ip:ip + 1])
                        else:
                            nc.vector.scalar_tensor_tensor(
                                out=pw1[:, ip, :], in0=pw[:, ip, :],
                                scalar=1.0, in1=pw[:, ip, :],
                                op0=Alu.bypass, op1=Alu.mult,
                                accum_out=s2[:, ip:ip + 1])

                    # mean = s1'/C + sum(bias)/C ; var = s2/C - mean^2
                    mu = small.tile([128, PG], f32, tag="mu")
                    nc.vector.scalar_tensor_tensor(
                        out=mu[:], in0=s1[:], scalar=1.0 / C, in1=sumbC[:],
                        op0=Alu.mult, op1=Alu.add)
                    mumu = small.tile([128, PG], f32, tag="mumu")
                    nc.vector.tensor_mul(mumu[:], mu[:], mu[:])
                    var = small.tile([128, PG], f32, tag="var")
                    nc.vector.scalar_tensor_tensor(
                        out=var[:], in0=s2[:], scalar=1.0 / C,
                        in1=mumu[:], op0=Alu.mult, op1=Alu.subtract)

                    std = small.tile([128, PG], f32, tag="std")
                    nc.scalar.activation(std[:], var[:], Act.Sqrt,
                                         bias=eps_t[:])
                    rstd = small.tile([128, PG], f32, tag="rstd")
                    nc.vector.reciprocal(rstd[:], std[:])
                    nmr = small.tile([128, PG], f32, tag="nmr")
                    # nmr = -mean * rstd
                    nc.vector.scalar_tensor_tensor(
                        out=nmr[:], in0=mu[:], scalar=-1.0,
                        in1=rstd[:], op0=Alu.mult, op1=Alu.mult)

                    # y_ln = (pw - mean) * rstd  (per point: two scalars)
                    for ip in range(PG):
                        nc.vector.tensor_scalar(
                            out=pw[:, ip, :], in0=pw[:, ip, :],
                            scalar1=rstd[:, ip:ip + 1],
                            scalar2=nmr[:, ip:ip + 1],
                            op0=Alu.mult, op1=Alu.add)
                    if has_gamma:
                        for ip in range(PG):
                            nc.vector.tensor_mul(pw[:, ip, :], pw[:, ip, :], gB[:])
                    if has_beta:
                        for ip in range(PG):
                            nc.vector.tensor_add(pw[:, ip, :], pw[:, ip, :], bB2[:])

                    # z = f + y_ln ; h = relu(z)  (alternate add engine)
                    if ibt % 2 == 0:
                        nc.gpsimd.tensor_add(f[:], f[:], pw[:])
                    else:
                        nc.vector.tensor_add(f[:], f[:], pw[:])
                    h = work.tile([128, PG, C], f32, tag="h")
                    nc.scalar.activation(h[:], f[:], Act.Relu)

                    # projection: out[d, b] = sum_c lin_w[d, c] * h[b, c].
                    # Transposes for 4 points share one PSUM bank per c-half,
                    # one batched copy to SBUF, then one N=512 matmul per half.
                    stage = stagep.tile([D, PG, 128], f32, tag="stage")
                    tb0 = psp.tile([128, 4, 128], f32, tag="tb")
                    tb1 = psp.tile([128, 4, 128], f32, tag="tb")
                    tb2 = psp.tile([128, 4, 128], f32, tag="tb")
                    for ip in range(4):
                        nc.tensor.transpose(tb0[:, ip, :], h[:, ip, 0:128],
                                            ident[:])
                        nc.tensor.transpose(tb1[:, ip, :], h[:, ip, 128:256],
                                            ident[:])
                    nc.tensor.transpose(tb2[:, 0, :], h[:, 4, 0:128], ident[:])
                    nc.tensor.transpose(tb2[:, 1, :], h[:, 4, 128:256], ident[:])
                    hTs0 = hTp.tile([128, 4, 128], f32, tag="hT")
                    hTs1 = hTp.tile([128, 4, 128], f32, tag="hT")
                    hTs2 = hTp.tile([128, 2, 128], f32, tag="hT2")
                    nc.scalar.copy(hTs0[:], tb0[:])
                    nc.scalar.copy(hTs1[:], tb1[:])
                    nc.scalar.copy(hTs2[:], tb2[:, 0:2, :])
                    ob0 = psop.tile([D, 4, 128], f32, tag="ob")
                    nc.tensor.matmul(ob0[:], lwT0[:], hTs0[:],
                                     start=True, stop=False)
                    nc.tensor.matmul(ob0[:], lwT1[:], hTs1[:],
                                     start=False, stop=True)
                    ob1 = psop.tile([D, 128], f32, tag="ob")
                    nc.tensor.matmul(ob1[:], lwT0[:], hTs2[:, 0, :],
                                     start=True, stop=False)
                    nc.tensor.matmul(ob1[:], lwT1[:], hTs2[:, 1, :],
                                     start=False, stop=True)
                    if has_linb:
                        nc.vector.tensor_scalar_add(
                            stage[:, 0:4, :], ob0[:], lbt[:])
                        nc.vector.tensor_scalar_add(
                            stage[:, 4, :], ob1[:], lbt[:])
                    else:
                        nc.scalar.copy(stage[:, 0:4, :], ob0[:])
                        nc.vector.tensor_copy(stage[:, 4, :], ob1[:])

                    nc.scalar.dma_start(
                        out=bass.AP(y, p0 * BS + b0,
                                    [[P * BS, D], [BS, PG], [1, 128]]),
                        in_=stage[:])

    nc.compile()
    return nc


def _get_nc(has_gamma, has_beta, has_linb):
    key = (has_gamma, has_beta, has_linb)
    if key not in _cache:
        _cache[key] = _build(*key)
    return _cache[key]


def kernel(features, weights, bias, ln_gamma, ln_beta, lin_w, lin_b):
    from concourse.bass_utils import run_bass_kernel_spmd

    features = np.asarray(features, dtype=np.float32)
    weights = np.asarray(weights, dtype=np.float32)
    bias = np.asarray(bias, dtype=np.float32)
    ln_gamma = np.asarray(ln_gamma, dtype=np.float32)
    ln_beta = np.asarray(ln_beta, dtype=np.float32)
    lin_w = np.asarray(lin_w, dtype=np.float32)
    lin_b = np.asarray(lin_b, dtype=np.float32)

    has_gamma = not np.all(ln_gamma == 1.0)
    has_beta = not np.all(ln_beta == 0.0)
    has_linb = not np.all(lin_b == 0.0)
    nc = _get_nc(has_gamma, has_beta, has_linb)

    # host-side resharding: [B, P, C] -> [P, B, C], slice batch per core
    xt = np.ascontiguousarray(features.transpose(1, 0, 2))
    wflat = np.ascontiguousarray(weights.reshape(P * C, C))
    in_maps = []
    for i in range(NCORES):
        in_maps.append({
            "x": np.ascontiguousarray(xt[:, i * BS:(i + 1) * BS, :]),
            "w": wflat,
            "bi": bias,
            "g": ln_gamma,
            "bt": ln_beta,
            "lw": lin_w,
            "lb": lin_b,
        })

    res = run_bass_kernel_spmd(nc, in_maps, core_ids=list(range(NCORES)))
    out = np.empty((B, P, D), dtype=np.float32)
    for i in range(NCORES):
        # y: [D, P, BS] -> [BS, P, D]
        out[i * BS:(i + 1) * BS] = res.results[i]["y"].transpose(2, 1, 0)
    return out


# revision 50
# speedup vs baseline: 1.1974x; 1.1974x over previous
"""Trainium2 Bass kernel for nn_DiseaseHead.

Computation (per the reference):
    w_rs = weights.sum(-1)                       # [P, C]
    pw   = features * w_rs + bias                # [B, P, C]
    pwn  = LayerNorm_c(pw) * gamma + beta
    h    = relu(features + pwn)
    out  = einsum("bpc,dc->bpd", h, lin_w) + lin_b

Sharding: data-parallel over batch B across 8 NeuronCores; all small
parameters replicated.  Device layout puts batch rows on SBUF partitions
(128 at a time) and groups PG=5 points per vector op so the free dim is
PG*C=1280.  LN stats via bn_stats/bn_aggr; normalize+residual+relu fused
into tensor_scalar + tensor_tensor + ACT relu; the D=5 projection runs on
the tensor engine after a PE transpose of h.
"""

import sys

if "/opt/trn_rl_repo" not in sys.path:
    sys.path.insert(0, "/opt/trn_rl_repo")

import numpy as np

B, P, C, D = 8192, 25, 256, 5
NCORES = 8
BS = B // NCORES          # 1024 batch rows per core
PG = 5                    # points per group
NPG = P // PG             # 5 groups
NBT = BS // 128           # 8 batch tiles of 128 rows
LN_EPS = 1e-5

_cache = {}


def _build(has_gamma: bool, has_beta: bool, has_linb: bool):
    import concourse.bass as bass
    import concourse.tile as tile
    from concourse import bacc, mybir
    from concourse.masks import make_identity

    f32 = mybir.dt.float32
    Alu = mybir.AluOpType
    Act = mybir.ActivationFunctionType

    nc = bacc.Bacc("TRN2", target_bir_lowering=False, debug=False,
                   num_devices=NCORES)

    x = nc.dram_tensor("x", [P, BS, C], f32, kind="ExternalInput")
    w = nc.dram_tensor("w", [P * C, C], f32, kind="ExternalInput")
    bi = nc.dram_tensor("bi", [P, C], f32, kind="ExternalInput")
    g = nc.dram_tensor("g", [C], f32, kind="ExternalInput")
    bt_ = nc.dram_tensor("bt", [C], f32, kind="ExternalInput")
    lw = nc.dram_tensor("lw", [D, C], f32, kind="ExternalInput")
    lb = nc.dram_tensor("lb", [D], f32, kind="ExternalInput")
    y = nc.dram_tensor("y", [D, P, BS], f32, kind="ExternalOutput")

    with tile.TileContext(nc) as tc:
        with (
            tc.tile_pool(name="consts", bufs=1) as consts,
            tc.tile_pool(name="wred", bufs=3) as wred,
            tc.tile_pool(name="bcast", bufs=2) as bcast,
            tc.tile_pool(name="feat", bufs=5) as feat,
            tc.tile_pool(name="work", bufs=5) as work,
            tc.tile_pool(name="small", bufs=10) as small,
            tc.tile_pool(name="hT", bufs=8) as hTp,
            tc.tile_pool(name="stage", bufs=6) as stagep,
            tc.tile_pool(name="ps", bufs=5, space="PSUM") as psp,
            tc.tile_pool(name="pso", bufs=3, space="PSUM") as psop,
            tc.tile_pool(name="dram", bufs=1, space="DRAM") as dramp,
        ):
            ident = consts.tile([128, 128], f32)
            make_identity(nc, ident[:])

            eps_t = consts.tile([128, 1], f32)
            nc.vector.memset(eps_t[:], LN_EPS)

            # lin_w^T halves: [K=c(128), M=d] stationaries
            lwT0 = consts.tile([128, D], f32)
            lwT1 = consts.tile([128, D], f32)
            nc.gpsimd.dma_start(
                out=lwT0[:], in_=bass.AP(lw, 0, [[1, 128], [C, D]]))
            nc.gpsimd.dma_start(
                out=lwT1[:], in_=bass.AP(lw, 128, [[1, 128], [C, D]]))

            if has_linb:
                lbt = consts.tile([D, 1], f32)
                nc.gpsimd.dma_start(out=lbt[:], in_=bass.AP(lb, 0, [[1, D], [1, 1]]))

            if has_gamma:
                gB = consts.tile([128, C], f32)
                nc.gpsimd.dma_start(out=gB[:], in_=bass.AP(g, 0, [[0, 128], [1, C]]))
            if has_beta:
                bB2 = consts.tile([128, C], f32)
                nc.gpsimd.dma_start(out=bB2[:], in_=bass.AP(bt_, 0, [[0, 128], [1, C]]))

            # ---- weights row-sum on device: w_rs[p, c] = sum_k w[p, c, k] ----
            NW = (P * C) // 128  # 50 row-tiles, loaded 10 at a time
            wcols = consts.tile([128, NW], f32)
            for j in range(0, NW, 10):
                wt = wred.tile([128, 10, C], f32)
                nc.default_dma_engine.dma_start(
                    out=wt[:],
                    in_=bass.AP(w, j * 128 * C,
                                [[C, 128], [128 * C, 10], [1, C]]))
                nc.vector.tensor_reduce(
                    out=wcols[:, j:j + 10], in_=wt[:],
                    axis=mybir.AxisListType.X, op=Alu.add)
            # transpose [128, NW] -> [NW, 128] and park in DRAM so stride-0
            # broadcast DMAs can replicate rows across partitions
            wrs_ps = psp.tile([NW, 128], f32, tag="tb")
            nc.tensor.transpose(wrs_ps[:], wcols[:], ident[:])
            wrs_sb = consts.tile([NW, 128], f32)
            nc.vector.tensor_copy(wrs_sb[:], wrs_ps[:])
            wrs_d = dramp.tile([NW * 128], f32)   # flat == [P, C] row-major
            nc.gpsimd.dma_start(out=wrs_d[:], in_=wrs_sb[:])
            wrs_ap = wrs_d[:]

            # ---- main loop, software-pipelined ----
            # Each engine executes its queue in program order, so a straight
            # per-iteration emission head-of-line-blocks DVE on mid-chain
            # POOL/ACT results.  Emit the body as 4 skewed stages so every
            # engine's queue interleaves work from different iterations.
            iters = [(pg, ibt) for pg in range(NPG) for ibt in range(NBT)]
            st = {}
            pgres = {}

            def load_pg(pg):
                p0 = pg * PG
                wB = bcast.tile([128, PG, C], f32, tag="wB")
                nc.gpsimd.dma_start(
                    out=wB[:],
                    in_=bass.AP(wrs_ap.tensor, wrs_ap.offset + p0 * C,
                                [[0, 128], [C, PG], [1, C]]))
                bB = bcast.tile([128, PG, C], f32, tag="bB")
                nc.gpsimd.dma_start(
                    out=bB[:],
                    in_=bass.AP(bi, p0 * C, [[0, 128], [C, PG], [1, C]]))
                # per-point bias sums for the mean correction: sum_c bias / C
                sumbC = bcast.tile([128, PG], f32, tag="sumbC")
                nc.vector.tensor_reduce(
                    out=sumbC[:], in_=bB[:], axis=mybir.AxisListType.X,
                    op=Alu.add)
                nc.vector.tensor_scalar_mul(sumbC[:], sumbC[:], 1.0 / C)
                pgres[pg] = (wB, bB, sumbC)
                if pg - 2 in pgres:
                    del pgres[pg - 2]

            def stage1(i):
                pg, ibt = iters[i]
                if pg not in pgres:
                    load_pg(pg)
                wB, bB, _ = pgres[pg]
                p0, b0 = pg * PG, ibt * 128
                f = feat.tile([128, PG, C], f32, tag="f")
                nc.default_dma_engine.dma_start(
                    out=f[:],
                    in_=bass.AP(x, (p0 * BS + b0) * C,
                                [[C, 128], [BS * C, PG], [1, C]]))
                # pw1 = f * w_rs with s1' = sum_c(f*w_rs) accumulated on the
                # multiply (DVE); bias-add on GPSIMD
                pw1 = work.tile([128, PG, C], f32, tag="pw1")
                s1 = small.tile([128, PG], f32, tag="s1")
                for ip in range(PG):
                    nc.vector.scalar_tensor_tensor(
                        out=pw1[:, ip, :], in0=f[:, ip, :], scalar=1.0,
                        in1=wB[:, ip, :], op0=Alu.bypass, op1=Alu.mult,
                        accum_out=s1[:, ip:ip + 1])
                pw = work.tile([128, PG, C], f32, tag="pw")
                nc.gpsimd.tensor_add(pw[:], pw1[:], bB[:])
                st[i] = {"f": f, "pw1": pw1, "pw": pw, "s1": s1, "pg": pg,
                         "ibt": ibt}

            def stage2(i):
                s = st[i]
                pw1, pw, s1 = s["pw1"], s["pw"], s["s1"]
                sumbC = pgres[s["pg"]][2]
                # s2 = sum_c pw^2 via square passes split ACT/DVE; squares
                # overwrite dead pw1
                s2 = small.tile([128, PG], f32, tag="s2")
                for ip in range(PG):
                    if ip < 2:
                        nc.scalar.activation(
                            pw1[:, ip, :], pw[:, ip, :], Act.Square,
                            accum_out=s2[:, ip:ip + 1])
                    else:
                        nc.vector.scalar_tensor_tensor(
                            out=pw1[:, ip, :], in0=pw[:, ip, :],
                            scalar=1.0, in1=pw[:, ip, :],
                            op0=Alu.bypass, op1=Alu.mult,
                            accum_out=s2[:, ip:ip + 1])
                # mean = s1'/C + sum(bias)/C ; var = s2/C - mean^2
                mu = small.tile([128, PG], f32, tag="mu")
                nc.vector.scalar_tensor_tensor(
                    out=mu[:], in0=s1[:], scalar=1.0 / C, in1=sumbC[:],
                    op0=Alu.mult, op1=Alu.add)
                mumu = small.tile([128, PG], f32, tag="mumu")
                nc.vector.tensor_mul(mumu[:], mu[:], mu[:])
                var = small.tile([128, PG], f32, tag="var")
                nc.vector.scalar_tensor_tensor(
                    out=var[:], in0=s2[:], scalar=1.0 / C,
                    in1=mumu[:], op0=Alu.mult, op1=Alu.subtract)
                std = small.tile([128, PG], f32, tag="std")
                nc.scalar.activation(std[:], var[:], Act.Sqrt, bias=eps_t[:])
                s["mu"], s["std"] = mu, std

            def stage3(i):
                s = st[i]
                f, pw, mu, std = s["f"], s["pw"], s["mu"], s["std"]
                rstd = small.tile([128, PG], f32, tag="rstd")
                nc.vector.reciprocal(rstd[:], std[:])
                nmr = small.tile([128, PG], f32, tag="nmr")
                # nmr = -mean * rstd
                nc.vector.scalar_tensor_tensor(
                    out=nmr[:], in0=mu[:], scalar=-1.0,
                    in1=rstd[:], op0=Alu.mult, op1=Alu.mult)
                # y_ln = (pw - mean) * rstd  (per point: two scalars)
                for ip in range(PG):
                    nc.vector.tensor_scalar(
                        out=pw[:, ip, :], in0=pw[:, ip, :],
                        scalar1=rstd[:, ip:ip + 1],
                        scalar2=nmr[:, ip:ip + 1],
                        op0=Alu.mult, op1=Alu.add)
                if has_gamma:
                    for ip in range(PG):
                        nc.vector.tensor_mul(pw[:, ip, :], pw[:, ip, :], gB[:])
                if has_beta:
                    for ip in range(PG):
                        nc.vector.tensor_add(pw[:, ip, :], pw[:, ip, :], bB2[:])
                # z = f + y_ln (alternate add engine); ReLU rides the
                # PSUM->SBUF copy in stage4
                if s["ibt"] % 2 == 0:
                    nc.gpsimd.tensor_add(f[:], f[:], pw[:])
                else:
                    nc.vector.tensor_add(f[:], f[:], pw[:])

            def stage4(i):
                s = st[i]
                f = s["f"]
                pg, ibt = s["pg"], s["ibt"]
                p0, b0 = pg * PG, ibt * 128
                # projection: out[d,b] = sum_c lin_w[d,c] * relu(z)[b,c].
                # Transposes for 4 points share one PSUM bank per c-half,
                # one batched Relu-copy to SBUF, one N=512 matmul per half.
                stage = stagep.tile([D, PG, 128], f32, tag="stage")
                tb0 = psp.tile([128, 4, 128], f32, tag="tb")
                tb1 = psp.tile([128, 4, 128], f32, tag="tb")
                tb2 = psp.tile([128, 4, 128], f32, tag="tb")
                for ip in range(4):
                    nc.tensor.transpose(tb0[:, ip, :], f[:, ip, 0:128],
                                        ident[:])
                    nc.tensor.transpose(tb1[:, ip, :], f[:, ip, 128:256],
                                        ident[:])
                nc.tensor.transpose(tb2[:, 0, :], f[:, 4, 0:128], ident[:])
                nc.tensor.transpose(tb2[:, 1, :], f[:, 4, 128:256], ident[:])
                hTs0 = hTp.tile([128, 4, 128], f32, tag="hT")
                hTs1 = hTp.tile([128, 4, 128], f32, tag="hT")
                hTs2 = hTp.tile([128, 2, 128], f32, tag="hT2")
                nc.scalar.activation(hTs0[:], tb0[:], Act.Relu)
                nc.scalar.activation(hTs1[:], tb1[:], Act.Relu)
                nc.scalar.activation(hTs2[:], tb2[:, 0:2, :], Act.Relu)
                ob0 = psop.tile([D, 4, 128], f32, tag="ob")
                nc.tensor.matmul(ob0[:], lwT0[:], hTs0[:],
                                 start=True, stop=False)
                nc.tensor.matmul(ob0[:], lwT1[:], hTs1[:],
                                 start=False, stop=True)
                ob1 = psop.tile([D, 128], f32, tag="ob")
                nc.tensor.matmul(ob1[:], lwT0[:], hTs2[:, 0, :],
                                 start=True, stop=False)
                nc.tensor.matmul(ob1[:], lwT1[:], hTs2[:, 1, :],
                                 start=False, stop=True)
                if has_linb:
                    nc.vector.tensor_scalar_add(
                        stage[:, 0:4, :], ob0[:], lbt[:])
                    nc.vector.tensor_scalar_add(
                        stage[:, 4, :], ob1[:], lbt[:])
                else:
                    nc.scalar.copy(stage[:, 0:4, :], ob0[:])
                    nc.vector.tensor_copy(stage[:, 4, :], ob1[:])
                nc.scalar.dma_start(
                    out=bass.AP(y, p0 * BS + b0,
                                [[P * BS, D], [BS, PG], [1, 128]]),
                    in_=stage[:])
                del st[i]

            n = len(iters)
            for k in range(n + 3):
                if k < n:
                    stage1(k)
                if 0 <= k - 1 < n:
                    stage2(k - 1)
                if 0 <= k - 2 < n:
                    stage3(k - 2)
                if 0 <= k - 3 < n:
                    stage4(k - 3)

    nc.compile()
    return nc


def _get_nc(has_gamma, has_beta, has_linb):
    key = (has_gamma, has_beta, has_linb)
    if key not in _cache:
        _cache[key] = _build(*key)
    return _cache[key]


def kernel(features, weights, bias, ln_gamma, ln_beta, lin_w, lin_b):
    from concourse.bass_utils import run_bass_kernel_spmd

    features = np.asarray(features, dtype=np.float32)
    weights = np.asarray(weights, dtype=np.float32)
    bias = np.asarray(bias, dtype=np.float32)
    ln_gamma = np.asarray(ln_gamma, dtype=np.float32)
    ln_beta = np.asarray(ln_beta, dtype=np.float32)
    lin_w = np.asarray(lin_w, dtype=np.float32)
    lin_b = np.asarray(lin_b, dtype=np.float32)

    has_gamma = not np.all(ln_gamma == 1.0)
    has_beta = not np.all(ln_beta == 0.0)
    has_linb = not np.all(lin_b == 0.0)
    nc = _get_nc(has_gamma, has_beta, has_linb)

    # host-side resharding: [B, P, C] -> [P, B, C], slice batch per core
    xt = np.ascontiguousarray(features.transpose(1, 0, 2))
    wflat = np.ascontiguousarray(weights.reshape(P * C, C))
    in_maps = []
    for i in range(NCORES):
        in_maps.append({
            "x": np.ascontiguousarray(xt[:, i * BS:(i + 1) * BS, :]),
            "w": wflat,
            "bi": bias,
            "g": ln_gamma,
            "bt": ln_beta,
            "lw": lin_w,
            "lb": lin_b,
        })

    res = run_bass_kernel_spmd(nc, in_maps, core_ids=list(range(NCORES)))
    out = np.empty((B, P, D), dtype=np.float32)
    for i in range(NCORES):
        # y: [D, P, BS] -> [BS, P, D]
        out[i * BS:(i + 1) * BS] = res.results[i]["y"].transpose(2, 1, 0)
    return out
